# revision 31
# baseline (speedup 1.0000x reference)
"""Trainium2 Bass kernel for a seq2seq LSTM (1-step encoder + T-step decoder + FC).

Model (B=512, I=256, H=1024, O=128, T=100):
  h,c   = LSTMCell(x, 0, 0; enc_Wih, enc_Whh, enc_b)          # encoder
  loop t in 0..T-1:  h,c = LSTMCell(dec_in, h, c; dec_*)      # decoder
      where dec_in == 0 for t==0 and dec_in == h (same tensor!) for t>=1
  out[:, t, :] = h_t @ fc_W.T + fc_b

Key algebraic fusion: for t>=1 the cell input equals the hidden state, so
  gates_t = h_{t-1} @ (dec_Wih + dec_Whh).T + dec_b
and for t==0 (dec_in = 0):
  gates_0 = h_enc @ dec_Whh.T + dec_b

Sharding: pure data-parallel over batch across 8 NeuronCores (64 rows each),
weights replicated.

Performance design (v2): the TRN2 PE clock p-states run at 1.2 GHz until the
engine has executed continuously for ~3 us, reaching 2.4 GHz only while it
stays busy; any long idle gap resets it.  The per-step schedule therefore
keeps the PE stream gapless:
  - matmuls: out = lhsT.T @ rhs with lhsT = transposed hidden state
    (hT, [128 x 64] per 128-hidden chunk), rhs = pre-transposed weights.
    PE column-pair tiling: two concurrent M=64 matmuls at (0,0)/(0,64)
    compute the two hidden-halves of each gate quarter into a folded
    [128, 512] PSUM tile (batch duplicated across partition halves).
  - each quarter's bias is ONE K=2 matmul (indicator-rows lhsT) opening the
    PSUM accumulation group.
  - the f and o quarters are split into two N=256 accumulation groups so the
    c-chain (sig f -> f*c + i*g -> tanh -> h) pipelines in column halves.
  - PE order per step: g,i,fA,fB,oA,oB matmuls | fc(t-1) | tr01 | bias_g(t+1)
    | tr23 | remaining biases(t+1) | next step.  The fc ride + biases fill
    the ACT/DVE tail; transposes run as soon as each h half lands; gate
    k-order {0,4,1,5,2,6,3,7} consumes the two hT copies as they complete.
"""

import os
import sys

import numpy as np

_TRN_REPO = "/opt/trn_rl_repo"
if _TRN_REPO not in sys.path:
    sys.path.insert(0, _TRN_REPO)

B, I, H, O, T = 512, 256, 1024, 128, 100
N_CORES = 8
BQ = B // N_CORES  # 64 batch rows per core
KCH = H // 128     # 8 k-chunks of the hidden dim
G4 = 4 * H         # 4096 gate columns
WALL_N = G4 + O    # gate weights + fc weights, concatenated along columns

_F32 = np.float32

# gate k-chunk order: consume hT transpose blocks 0,1 then 2,3
K_ORDER = (0, 4, 1, 5, 2, 6, 3, 7)


def _bf16(a):
    import ml_dtypes

    return np.asarray(a, dtype=ml_dtypes.bfloat16)


_CALIBRATED = False


def _calibrate_cost_model():
    """Calibrate the build-time cost model's ACT/DVE fixed overheads to the
    values measured on hardware (the stock spec is ~150-250 ns/op optimistic
    for the Activation engine).  The Tile scheduler freezes each engine's
    instruction order against this model, and enforces that order with
    cross-engine waits; an optimistic ACT model makes the frozen PE order
    stall on activations that finish later than predicted.  Must run before
    the first compile in the process (the Rust cost model caches the spec).
    """
    global _CALIBRATED
    if _CALIBRATED:
        return
    _CALIBRATED = True
    import concourse.bass as bass
    from concourse import hw_specs, mybir

    ac = dict(hw_specs.TRN2Spec.ACCESS_CYCLES)
    ac[(bass.MemorySpace.SBUF, mybir.EngineType.Activation)] = 470
    ac[(bass.MemorySpace.PSUM, mybir.EngineType.Activation)] = 420
    ac[(bass.MemorySpace.PSUM, mybir.EngineType.DVE)] = 160
    hw_specs.TRN2Spec.ACCESS_CYCLES = ac


def build_bass(T_steps=T, tiny_out=False):
    """Builds the per-core Bass program (same program on all 8 cores)."""
    import concourse.bass as bass
    import concourse.tile as tile
    from concourse import bacc, mybir

    _calibrate_cost_model()

    f32 = mybir.dt.float32
    bf16 = mybir.dt.bfloat16
    AF = mybir.ActivationFunctionType

    nc = bacc.Bacc("TRN2", target_bir_lowering=False, debug=False,
                   enable_asserts=False)

    # ---- DRAM I/O ----
    xT_d = nc.dram_tensor("xT", [I, BQ], bf16, kind="ExternalInput").ap()
    encW_d = nc.dram_tensor("encW", [I, G4], bf16, kind="ExternalInput").ap()
    whhT_d = nc.dram_tensor("whhT", [H, G4], bf16, kind="ExternalInput").ap()
    wall_d = nc.dram_tensor("wall", [H, WALL_N], bf16, kind="ExternalInput").ap()
    encb2_d = nc.dram_tensor("encb2", [2, G4 // 2], bf16, kind="ExternalInput").ap()
    decb2_d = nc.dram_tensor("decb2", [2, G4 // 2], bf16, kind="ExternalInput").ap()
    ones_d = nc.dram_tensor("ones", [1, BQ], bf16, kind="ExternalInput").ap()
    fold2_d = nc.dram_tensor("fold2", [2, 128], bf16, kind="ExternalInput").ap()
    ident_d = nc.dram_tensor("ident", [128, 128], bf16, kind="ExternalInput").ap()
    out_T = 1 if tiny_out else T_steps
    out_d = nc.dram_tensor("out", [BQ, out_T, O], f32, kind="ExternalOutput").ap()

    QH = 512  # hidden half (columns per folded tile)
    QA = 256  # split-quarter group width

    with tile.TileContext(nc) as tc:
        from contextlib import ExitStack

        ctx = ExitStack()
        with ctx:
            # ---- persistent SBUF pools ----
            consts = ctx.enter_context(tc.tile_pool(name="consts", bufs=1))
            wpool = ctx.enter_context(tc.tile_pool(name="wpool", bufs=1))
            wtmp = ctx.enter_context(tc.tile_pool(name="wtmp", bufs=2))
            cpool = ctx.enter_context(tc.tile_pool(name="cpool", bufs=2))
            hpool = ctx.enter_context(tc.tile_pool(name="hpool", bufs=2))
            htpool = ctx.enter_context(tc.tile_pool(name="htpool", bufs=2))
            sgpool = ctx.enter_context(tc.tile_pool(name="sgpool", bufs=2))
            ttpool = ctx.enter_context(tc.tile_pool(name="ttpool", bufs=2))
            fcpool = ctx.enter_context(tc.tile_pool(name="fcpool", bufs=3))
            # PSUM pools (8 banks total: 3 + 2 + 1 + 2).  The o quarter
            # gets its own 2-buffer pool: its bias (opened in the step tail)
            # must never WAR-wait on the just-completed sig_o of this step.
            pg = ctx.enter_context(tc.tile_pool(name="pg", bufs=3, space="PSUM"))
            po = ctx.enter_context(tc.tile_pool(name="po", bufs=2, space="PSUM"))
            ptr = ctx.enter_context(tc.tile_pool(name="ptr", bufs=1, space="PSUM"))
            pfc = ctx.enter_context(tc.tile_pool(name="pfc", bufs=2, space="PSUM"))

            # ---- constants / weights into SBUF ----
            ones_sb = consts.tile([1, BQ], bf16, tag="ones")
            nc.sync.dma_start(ones_sb[:], ones_d[:])
            fold2_sb = consts.tile([2, 128], bf16, tag="fold2")
            nc.sync.dma_start(fold2_sb[:], fold2_d[:])
            ident_sb = consts.tile([128, 128], bf16, tag="ident")
            nc.sync.dma_start(ident_sb[:], ident_d[:])
            encb2_sb = consts.tile([2, G4 // 2], bf16, tag="encb2")
            nc.sync.dma_start(encb2_sb[:], encb2_d[:])
            decb2_sb = consts.tile([2, G4 // 2], bf16, tag="decb2")
            nc.sync.dma_start(decb2_sb[:], decb2_d[:])
            xT_sb = consts.tile([128, 2 * BQ], bf16, tag="xT")
            for k in range(2):
                nc.sync.dma_start(xT_sb[:, k * BQ:(k + 1) * BQ],
                                  xT_d[k * 128:(k + 1) * 128, :])
            encW_sb = consts.tile([128, 2 * G4], bf16, tag="encW")
            for k in range(2):
                nc.sync.dma_start(encW_sb[:, k * G4:(k + 1) * G4],
                                  encW_d[k * 128:(k + 1) * 128, :])

            # main fused weights, resident: [128, KCH * WALL_N] bf16
            # (issued on the gpsimd queue so the whhT stream on the sync
            # queue is not serialized behind it)
            wall_sb = wpool.tile([128, KCH * WALL_N], bf16, tag="wall")
            for k in range(KCH):
                nc.gpsimd.dma_start(wall_sb[:, k * WALL_N:(k + 1) * WALL_N],
                                    wall_d[k * 128:(k + 1) * 128, :])

            def wall_rhs(k, col0, ncols):
                return wall_sb[:, k * WALL_N + col0: k * WALL_N + col0 + ncols]

            # ---------------- helpers ----------------
            QI, QF, QG, QO = 0, 1, 2, 3

            def bias_mm(p, q, bias2_sb, col0=0, ncols=QH):
                """One K=2 matmul: the indicator-rows lhsT broadcasts the
                per-half gate bias rows into the folded [128, ncols] PSUM
                region, opening its accumulation group."""
                nc.tensor.matmul(
                    p[:, col0:col0 + ncols],
                    fold2_sb[:, :],
                    bias2_sb[:, q * QH + col0: q * QH + col0 + ncols],
                    start=True, stop=False,
                    skip_group_check=True,
                )

            def quarter_mms(p, q, lhs_fn, rhs_fn, kseq, col0=0, ncols=QH,
                            want_stop=True):
                """Gate-quarter matmul pairs into folded PSUM columns
                [col0:col0+ncols]; group must already be open (bias_mm).
                want_stop=False leaves the group open (sub-range splits:
                only the final sub-range's last matmul carries stop)."""
                last = kseq[-1]
                for k in kseq:
                    for half in range(2):
                        c0 = q * H + half * QH + col0
                        nc.tensor.matmul(
                            p[half * 64:(half + 1) * 64, col0:col0 + ncols],
                            lhs_fn(k),
                            rhs_fn(k, c0, ncols),
                            start=False,
                            stop=(want_stop and k == last),
                            skip_group_check=True,
                        )

            def act(func, out_t, in_t):
                nc.scalar.activation(out_t[:], in_t[:], func)

            def act_cols(func, out_t, in_t, c0, nc_):
                nc.scalar.activation(out_t[:, c0:c0 + nc_], in_t[:, c0:c0 + nc_],
                                     func)

            def ht_chunk(hT, k):
                col0 = (k % 4) * 128 + (k // 4) * 64
                return hT[:, col0:col0 + 64]

            def fc_mms(hT_prev):
                # fc bias is added on the host after the gather
                p = pfc.tile([64, O], f32, tag="pfc")
                for k in range(KCH):
                    nc.tensor.matmul(
                        p[:, :],
                        ht_chunk(hT_prev, k),
                        wall_rhs(k, G4, O),
                        start=(k == 0), stop=(k == KCH - 1),
                    )
                return p

            def fc_out(p, t_row):
                sb = fcpool.tile([64, O], f32, tag="fcsb")
                nc.vector.tensor_copy(sb[:], p[:])
                if tiny_out:
                    if t_row == T_steps - 1:
                        nc.sync.dma_start(out_d[:, 0, :], sb[:])
                else:
                    nc.sync.dma_start(out_d[:, t_row, :], sb[:])

            def transpose_blocks(h_both, p, blocks):
                """PE transposes of folded-h column blocks into ptr PSUM.
                Block j covers h cols [j*128,(j+1)*128) -> hT chunks j, j+4."""
                for j in blocks:
                    nc.tensor.matmul(
                        p[:, j * 128:(j + 1) * 128],
                        h_both[:, j * 128:(j + 1) * 128],
                        ident_sb[:, :],
                        is_transpose=True, start=True, stop=True,
                    )

            # ---------------- encoder ----------------
            # gates_e = x @ enc_Wih.T + enc_b ; f-gate unused (c_prev = 0)
            def enc_lhs(k):
                return xT_sb[:, k * BQ:(k + 1) * BQ]

            def enc_rhs(k, col0, ncols):
                return encW_sb[:, k * G4 + col0: k * G4 + col0 + ncols]

            p_g = pg.tile([128, QH], f32, tag="pg", name="enc_g")
            bias_mm(p_g, QG, encb2_sb)
            quarter_mms(p_g, QG, enc_lhs, enc_rhs, (0, 1))
            s_g = sgpool.tile([128, QH], f32, tag="s_g")
            act(AF.Tanh, s_g, p_g)
            p_i = pg.tile([128, QH], f32, tag="pg", name="enc_i")
            bias_mm(p_i, QI, encb2_sb)
            quarter_mms(p_i, QI, enc_lhs, enc_rhs, (0, 1))
            s_i = sgpool.tile([128, QH], f32, tag="s_i")
            act(AF.Sigmoid, s_i, p_i)
            c_cur = cpool.tile([128, QH], f32, tag="c")
            nc.vector.tensor_mul(c_cur[:], s_i[:], s_g[:])
            tc_t = ttpool.tile([128, QH], f32, tag="tc")
            act(AF.Tanh, tc_t, c_cur)
            p_o = po.tile([128, QH], f32, tag="po", name="enc_o")
            bias_mm(p_o, QO, encb2_sb)
            quarter_mms(p_o, QO, enc_lhs, enc_rhs, (0, 1))
            s_o = sgpool.tile([128, QH], f32, tag="s_o")
            act(AF.Sigmoid, s_o, p_o)
            h_both = hpool.tile([128, QH], bf16, tag="h")
            nc.vector.tensor_mul(h_both[:], s_o[:], tc_t[:])
            p_tr = ptr.tile([128, KCH * 64], bf16, tag="ptr", name="enc_tr")
            transpose_blocks(h_both, p_tr, (0, 1, 2, 3))
            hT_cur = htpool.tile([128, KCH * 64], bf16, tag="hT",
                                 name="enc_hT")
            nc.vector.tensor_copy(hT_cur[:], p_tr[:])

            # ---------------- decoder step 0 ----------------
            # gates_0 = h_enc @ dec_Whh.T + dec_b, streaming whhT from HBM in
            # k-chunk pairs; k-outer so each streamed chunk is used once.
            hT_prev = hT_cur
            c_prev = c_cur
            pq = [(po if q_ == QO else pg).tile(
                [128, QH], f32, tag=("po" if q_ == QO else "pg"),
                name=f"t0_q{q_}") for q_ in range(4)]
            for q in range(4):
                bias_mm(pq[q], q, decb2_sb)
            for pair in range(KCH // 2):
                wt = wtmp.tile([128, 2 * G4], bf16, tag="wt")
                for kk in range(2):
                    k = 2 * pair + kk
                    nc.sync.dma_start(
                        wt[:, kk * G4:(kk + 1) * G4],
                        whhT_d[k * 128:(k + 1) * 128, :])
                for kk in range(2):
                    k = 2 * pair + kk
                    last = k == KCH - 1
                    for q in range(4):
                        for half in range(2):
                            col0 = q * H + half * QH
                            nc.tensor.matmul(
                                pq[q][half * 64:(half + 1) * 64, :],
                                ht_chunk(hT_prev, k),
                                wt[:, kk * G4 + col0: kk * G4 + col0 + QH],
                                start=False, stop=last,
                                skip_group_check=True,
                            )
            p_g, p_i, p_f, p_o = pq[QG], pq[QI], pq[QF], pq[QO]
            s_g = sgpool.tile([128, QH], f32, tag="s_g")
            act(AF.Tanh, s_g, p_g)
            s_i = sgpool.tile([128, QH], f32, tag="s_i")
            act(AF.Sigmoid, s_i, p_i)
            s_f = sgpool.tile([128, QH], f32, tag="s_f")
            act(AF.Sigmoid, s_f, p_f)
            t2 = ttpool.tile([128, QH], f32, tag="t2")
            nc.vector.tensor_mul(t2[:], s_i[:], s_g[:])
            t1 = ttpool.tile([128, QH], f32, tag="t1")
            nc.vector.tensor_mul(t1[:], s_f[:], c_prev[:])
            c_cur = cpool.tile([128, QH], f32, tag="c")
            nc.vector.tensor_add(c_cur[:], t1[:], t2[:])
            tc_t = ttpool.tile([128, QH], f32, tag="tc")
            act(AF.Tanh, tc_t, c_cur)
            s_o = sgpool.tile([128, QH], f32, tag="s_o")
            act(AF.Sigmoid, s_o, p_o)
            h_both = hpool.tile([128, QH], bf16, tag="h")
            nc.vector.tensor_mul(h_both[:], s_o[:], tc_t[:])
            p_tr = ptr.tile([128, KCH * 64], bf16, tag="ptr", name="t0_tr")
            transpose_blocks(h_both, p_tr, (0, 1, 2, 3))
            hT_cur = htpool.tile([128, KCH * 64], bf16, tag="hT",
                                 name="t0_hT")
            nc.vector.tensor_copy(hT_cur[:], p_tr[:])

            # ---------------- decoder steps 1..T-1 ----------------
            for t in range(1, T_steps):
                hT_prev = hT_cur
                c_prev = c_cur
                # allocate at step start in [g,i,f,o] order: vs the 5-slot
                # ring this gives g<-o(t-2), i<-g(t-1), f<-i(t-1), o<-f(t-1)
                # so no bias WARs on a late-read tile
                p_gq = pg.tile([128, QH], f32, tag="pg", name=f"s{t}_g")
                p_iq = pg.tile([128, QH], f32, tag="pg", name=f"s{t}_i")
                p_fq = pg.tile([128, QH], f32, tag="pg", name=f"s{t}_f")
                p_oq = po.tile([128, QH], f32, tag="po", name=f"s{t}_o")

                def dec_lhs(k, _h=hT_prev):
                    return ht_chunk(_h, k)

                # --- PE: gate quarters (g, i full; f, o split A/B) ---
                bias_mm(p_gq, QG, decb2_sb)
                quarter_mms(p_gq, QG, dec_lhs, wall_rhs, K_ORDER)
                bias_mm(p_iq, QI, decb2_sb)
                quarter_mms(p_iq, QI, dec_lhs, wall_rhs, K_ORDER)
                bias_mm(p_fq, QF, decb2_sb)
                bias_mm(p_oq, QO, decb2_sb)
                quarter_mms(p_fq, QF, dec_lhs, wall_rhs, K_ORDER, 0, QA,
                            want_stop=False)
                quarter_mms(p_fq, QF, dec_lhs, wall_rhs, K_ORDER, QA, QA)
                quarter_mms(p_oq, QO, dec_lhs, wall_rhs, K_ORDER, 0, QA,
                            want_stop=False)
                quarter_mms(p_oq, QO, dec_lhs, wall_rhs, K_ORDER, QA, QA)

                # --- ACT queue: gate activations (explicit order) ---
                s_g = sgpool.tile([128, QH], f32, tag="s_g")
                act(AF.Tanh, s_g, p_gq)
                s_i = sgpool.tile([128, QH], f32, tag="s_i")
                act(AF.Sigmoid, s_i, p_iq)
                s_f = sgpool.tile([128, QH], f32, tag="s_f")
                act_cols(AF.Sigmoid, s_f, p_fq, 0, QA)
                act_cols(AF.Sigmoid, s_f, p_fq, QA, QA)
                s_o = sgpool.tile([128, QH], f32, tag="s_o")
                tc_t = ttpool.tile([128, QH], f32, tag="tc")
                t2 = ttpool.tile([128, QH], f32, tag="t2")
                t1 = ttpool.tile([128, QH], f32, tag="t1")
                c_cur = cpool.tile([128, QH], f32, tag="c")
                h_both = hpool.tile([128, QH], bf16, tag="h")
                # c-chain on DVE, activations on ACT
                nc.vector.tensor_mul(t2[:, 0:QA], s_i[:, 0:QA], s_g[:, 0:QA])
                nc.vector.tensor_mul(t1[:, 0:QA], s_f[:, 0:QA],
                                     c_prev[:, 0:QA])
                nc.vector.tensor_add(c_cur[:, 0:QA], t1[:, 0:QA], t2[:, 0:QA])
                act_cols(AF.Tanh, tc_t, c_cur, 0, QA)
                act_cols(AF.Sigmoid, s_o, p_oq, 0, QA)
                nc.vector.tensor_mul(t2[:, QA:QH], s_i[:, QA:QH],
                                     s_g[:, QA:QH])
                nc.vector.tensor_mul(t1[:, QA:QH], s_f[:, QA:QH],
                                     c_prev[:, QA:QH])
                nc.vector.tensor_add(c_cur[:, QA:QH], t1[:, QA:QH],
                                     t2[:, QA:QH])
                act_cols(AF.Tanh, tc_t, c_cur, QA, QA)
                act_cols(AF.Sigmoid, s_o, p_oq, QA, QA)
                nc.vector.tensor_mul(h_both[:, 0:QA], s_o[:, 0:QA],
                                     tc_t[:, 0:QA])
                nc.vector.tensor_mul(h_both[:, QA:QH], s_o[:, QA:QH],
                                     tc_t[:, QA:QH])

                # --- PE tail: fc(t-1) ride, transposes, next-step biases ---
                p = fc_mms(hT_prev)
                p_tr = ptr.tile([128, KCH * 64], bf16, tag="ptr",
                                name=f"tr{t}")
                transpose_blocks(h_both, p_tr, (0, 1))

                transpose_blocks(h_both, p_tr, (2, 3))

                # hT copies: block 0 then block 1 on ACT (fine-grained so
                # the next step's first gate pairs unblock after ~1 block),
                # blocks 2,3 on DVE
                hT_cur = htpool.tile([128, KCH * 64], bf16, tag="hT",
                                     name=f"hT{t}")
                nc.scalar.activation(hT_cur[:, 0:128], p_tr[:, 0:128], AF.Copy)
                nc.scalar.activation(hT_cur[:, 128:256], p_tr[:, 128:256],
                                     AF.Copy)
                nc.vector.tensor_copy(hT_cur[:, 256:512], p_tr[:, 256:512])
                fc_out(p, t - 1)

            # fc epilogue for the last step's h
            p = fc_mms(hT_cur)
            fc_out(p, T_steps - 1)

    nc.compile()
    return nc


def _prep_inputs(x, enc_Wih, enc_Whh, enc_bih, enc_bhh,
                 dec_Wih, dec_Whh, dec_bih, dec_bhh, fc_W, fc_b):
    """Host-side prep: fuse/transpose/cast; returns per-core in_maps."""
    x = np.asarray(x, _F32)
    wc = np.asarray(dec_Wih, _F32) + np.asarray(dec_Whh, _F32)  # [4H, H]
    wall = np.concatenate([wc.T, np.asarray(fc_W, _F32).T], axis=1)  # [H, 4H+O]
    whhT = np.ascontiguousarray(np.asarray(dec_Whh, _F32).T)  # [H, 4H]
    encW = np.ascontiguousarray(np.asarray(enc_Wih, _F32).T)  # [I, 4H]
    encb = np.asarray(enc_bih, _F32) + np.asarray(enc_bhh, _F32)
    decb = np.asarray(dec_bih, _F32) + np.asarray(dec_bhh, _F32)

    def stack2(b):
        # [2, 2048]: row0 = per-quarter first halves, row1 = second halves
        q = b.reshape(4, 2, 512)
        return np.stack([q[:, 0, :].reshape(-1), q[:, 1, :].reshape(-1)])

    encb2 = stack2(encb)
    decb2 = stack2(decb)
    xT = np.ascontiguousarray(x.T)  # [I, B]
    ident = np.eye(128, dtype=_F32)
    ones = np.ones((1, BQ), _F32)
    fold2 = np.zeros((2, 128), _F32)
    fold2[0, 0:64] = 1.0
    fold2[1, 64:128] = 1.0

    shared = {
        "encW": _bf16(encW),
        "whhT": _bf16(whhT),
        "wall": _bf16(wall),
        "encb2": _bf16(encb2),
        "decb2": _bf16(decb2),
        "ones": _bf16(ones),
        "fold2": _bf16(fold2),
        "ident": _bf16(ident),
    }
    in_maps = []
    for c in range(N_CORES):
        m = dict(shared)
        m["xT"] = _bf16(xT[:, c * BQ:(c + 1) * BQ])
        in_maps.append(m)
    return in_maps


_CACHED = {}


def _get_compiled(T_steps=T):
    if T_steps not in _CACHED:
        _CACHED[T_steps] = build_bass(T_steps)
    return _CACHED[T_steps]


def kernel(**inputs):
    from concourse.bass_utils import run_bass_kernel_spmd

    nc = _get_compiled(T)
    in_maps = _prep_inputs(**inputs)
    res = run_bass_kernel_spmd(nc, in_maps, core_ids=list(range(N_CORES)))
    outs = [res.results[c]["out"] for c in range(N_CORES)]
    out = np.concatenate(outs, axis=0)  # [B, T, O] fp32
    out += np.asarray(inputs["fc_b"], _F32)[None, None, :]
    return out


if __name__ == "__main__":
    # quick shape smoke test with random inputs
    rng = np.random.default_rng(0)
    ins = {
        "x": rng.standard_normal((B, I), dtype=_F32),
        "enc_Wih": rng.standard_normal((G4, I), dtype=_F32) * 0.03,
        "enc_Whh": rng.standard_normal((G4, H), dtype=_F32) * 0.03,
        "enc_bih": rng.standard_normal(G4).astype(_F32) * 0.03,
        "enc_bhh": rng.standard_normal(G4).astype(_F32) * 0.03,
        "dec_Wih": rng.standard_normal((G4, H), dtype=_F32) * 0.03,
        "dec_Whh": rng.standard_normal((G4, H), dtype=_F32) * 0.03,
        "dec_bih": rng.standard_normal(G4).astype(_F32) * 0.03,
        "dec_bhh": rng.standard_normal(G4).astype(_F32) * 0.03,
        "fc_W": rng.standard_normal((O, H), dtype=_F32) * 0.03,
        "fc_b": rng.standard_normal(O).astype(_F32) * 0.03,
    }
    out = kernel(**ins)
    print("out", out.shape, out.dtype, float(np.abs(out).mean()))


# revision 32
# speedup vs baseline: 1.1914x; 1.1914x over previous
"""Trainium2 Bass kernel for a seq2seq LSTM (1-step encoder + T-step decoder + FC).

Model (B=512, I=256, H=1024, O=128, T=100):
  h,c   = LSTMCell(x, 0, 0; enc_Wih, enc_Whh, enc_b)          # encoder
  loop t in 0..T-1:  h,c = LSTMCell(dec_in, h, c; dec_*)      # decoder
      where dec_in == 0 for t==0 and dec_in == h (same tensor!) for t>=1
  out[:, t, :] = h_t @ fc_W.T + fc_b

Key algebraic fusion: for t>=1 the cell input equals the hidden state, so
  gates_t = h_{t-1} @ (dec_Wih + dec_Whh).T + dec_b
and for t==0 (dec_in = 0):
  gates_0 = h_enc @ dec_Whh.T + dec_b

Sharding: pure data-parallel over batch across 8 NeuronCores (64 rows each),
weights replicated.

Performance design (v2): the TRN2 PE clock p-states run at 1.2 GHz until the
engine has executed continuously for ~3 us, reaching 2.4 GHz only while it
stays busy; any long idle gap resets it.  The per-step schedule therefore
keeps the PE stream gapless:
  - matmuls: out = lhsT.T @ rhs with lhsT = transposed hidden state
    (hT, [128 x 64] per 128-hidden chunk), rhs = pre-transposed weights.
    PE column-pair tiling: two concurrent M=64 matmuls at (0,0)/(0,64)
    compute the two hidden-halves of each gate quarter into a folded
    [128, 512] PSUM tile (batch duplicated across partition halves).
  - each quarter's bias is ONE K=2 matmul (indicator-rows lhsT) opening the
    PSUM accumulation group.
  - the f and o quarters are split into two N=256 accumulation groups so the
    c-chain (sig f -> f*c + i*g -> tanh -> h) pipelines in column halves.
  - PE order per step: g,i,fA,fB,oA,oB matmuls | fc(t-1) | tr01 | bias_g(t+1)
    | tr23 | remaining biases(t+1) | next step.  The fc ride + biases fill
    the ACT/DVE tail; transposes run as soon as each h half lands; gate
    k-order {0,4,1,5,2,6,3,7} consumes the two hT copies as they complete.
"""

import os
import sys

import numpy as np

_TRN_REPO = "/opt/trn_rl_repo"
if _TRN_REPO not in sys.path:
    sys.path.insert(0, _TRN_REPO)

B, I, H, O, T = 512, 256, 1024, 128, 100
N_CORES = 8
BQ = B // N_CORES  # 64 batch rows per core
KCH = H // 128     # 8 k-chunks of the hidden dim
G4 = 4 * H         # 4096 gate columns
WALL_N = G4 + O    # gate weights + fc weights, concatenated along columns

_F32 = np.float32

# gate k-chunk order: consume hT transpose blocks 0,1 then 2,3
K_ORDER = (0, 4, 1, 5, 2, 6, 3, 7)


def _bf16(a):
    import ml_dtypes

    return np.asarray(a, dtype=ml_dtypes.bfloat16)


_CALIBRATED = False


def _calibrate_cost_model():
    """Calibrate the build-time cost model's ACT/DVE fixed overheads to the
    values measured on hardware (the stock spec is ~150-250 ns/op optimistic
    for the Activation engine).  The Tile scheduler freezes each engine's
    instruction order against this model, and enforces that order with
    cross-engine waits; an optimistic ACT model makes the frozen PE order
    stall on activations that finish later than predicted.  Must run before
    the first compile in the process (the Rust cost model caches the spec).
    """
    global _CALIBRATED
    if _CALIBRATED:
        return
    _CALIBRATED = True
    import concourse.bass as bass
    from concourse import hw_specs, mybir

    ac = dict(hw_specs.TRN2Spec.ACCESS_CYCLES)
    ac[(bass.MemorySpace.SBUF, mybir.EngineType.Activation)] = 470
    ac[(bass.MemorySpace.PSUM, mybir.EngineType.Activation)] = 420
    ac[(bass.MemorySpace.PSUM, mybir.EngineType.DVE)] = 160
    hw_specs.TRN2Spec.ACCESS_CYCLES = ac


def build_bass(T_steps=T, tiny_out=False):
    """Builds the per-core Bass program (same program on all 8 cores)."""
    import concourse.bass as bass
    import concourse.tile as tile
    from concourse import bacc, mybir

    _calibrate_cost_model()

    f32 = mybir.dt.float32
    bf16 = mybir.dt.bfloat16
    AF = mybir.ActivationFunctionType

    nc = bacc.Bacc("TRN2", target_bir_lowering=False, debug=False,
                   enable_asserts=False)

    # ---- DRAM I/O ----
    xT_d = nc.dram_tensor("xT", [I, BQ], bf16, kind="ExternalInput").ap()
    encW_d = nc.dram_tensor("encW", [I, G4], bf16, kind="ExternalInput").ap()
    whhT_d = nc.dram_tensor("whhT", [H, G4], bf16, kind="ExternalInput").ap()
    wall_d = nc.dram_tensor("wall", [H, WALL_N], bf16, kind="ExternalInput").ap()
    encb2_d = nc.dram_tensor("encb2", [2, G4 // 2], bf16, kind="ExternalInput").ap()
    decb2_d = nc.dram_tensor("decb2", [2, G4 // 2], bf16, kind="ExternalInput").ap()
    ones_d = nc.dram_tensor("ones", [1, BQ], bf16, kind="ExternalInput").ap()
    fold2_d = nc.dram_tensor("fold2", [2, 128], bf16, kind="ExternalInput").ap()
    ident_d = nc.dram_tensor("ident", [128, 128], bf16, kind="ExternalInput").ap()
    out_T = 1 if tiny_out else T_steps
    out_d = nc.dram_tensor("out", [BQ, out_T, O], f32, kind="ExternalOutput").ap()

    QH = 512  # hidden half (columns per folded tile)
    QA = 256  # split-quarter group width

    with tile.TileContext(nc) as tc:
        from contextlib import ExitStack

        ctx = ExitStack()
        with ctx:
            # ---- persistent SBUF pools ----
            consts = ctx.enter_context(tc.tile_pool(name="consts", bufs=1))
            wpool = ctx.enter_context(tc.tile_pool(name="wpool", bufs=1))
            wtmp = ctx.enter_context(tc.tile_pool(name="wtmp", bufs=2))
            cpool = ctx.enter_context(tc.tile_pool(name="cpool", bufs=2))
            hpool = ctx.enter_context(tc.tile_pool(name="hpool", bufs=2))
            htpool = ctx.enter_context(tc.tile_pool(name="htpool", bufs=2))
            sgpool = ctx.enter_context(tc.tile_pool(name="sgpool", bufs=2))
            ttpool = ctx.enter_context(tc.tile_pool(name="ttpool", bufs=2))
            fcpool = ctx.enter_context(tc.tile_pool(name="fcpool", bufs=3))
            # PSUM pools (8 banks total: 3 + 2 + 1 + 2).  The o quarter
            # gets its own 2-buffer pool: its bias (opened in the step tail)
            # must never WAR-wait on the just-completed sig_o of this step.
            pg = ctx.enter_context(tc.tile_pool(name="pg", bufs=3, space="PSUM"))
            po = ctx.enter_context(tc.tile_pool(name="po", bufs=2, space="PSUM"))
            ptr = ctx.enter_context(tc.tile_pool(name="ptr", bufs=1, space="PSUM"))
            pfc = ctx.enter_context(tc.tile_pool(name="pfc", bufs=2, space="PSUM"))

            # ---- constants / weights into SBUF ----
            ones_sb = consts.tile([1, BQ], bf16, tag="ones")
            nc.sync.dma_start(ones_sb[:], ones_d[:])
            fold2_sb = consts.tile([2, 128], bf16, tag="fold2")
            nc.sync.dma_start(fold2_sb[:], fold2_d[:])
            ident_sb = consts.tile([128, 128], bf16, tag="ident")
            nc.sync.dma_start(ident_sb[:], ident_d[:])
            encb2_sb = consts.tile([2, G4 // 2], bf16, tag="encb2")
            nc.sync.dma_start(encb2_sb[:], encb2_d[:])
            decb2_sb = consts.tile([2, G4 // 2], bf16, tag="decb2")
            nc.sync.dma_start(decb2_sb[:], decb2_d[:])
            xT_sb = consts.tile([128, 2 * BQ], bf16, tag="xT")
            for k in range(2):
                nc.sync.dma_start(xT_sb[:, k * BQ:(k + 1) * BQ],
                                  xT_d[k * 128:(k + 1) * 128, :])
            encW_sb = consts.tile([128, 2 * G4], bf16, tag="encW")
            for k in range(2):
                nc.sync.dma_start(encW_sb[:, k * G4:(k + 1) * G4],
                                  encW_d[k * 128:(k + 1) * 128, :])

            # main fused weights, resident: [128, KCH * WALL_N] bf16
            # (issued on the gpsimd queue so the whhT stream on the sync
            # queue is not serialized behind it)
            wall_sb = wpool.tile([128, KCH * WALL_N], bf16, tag="wall")
            for k in range(KCH):
                nc.gpsimd.dma_start(wall_sb[:, k * WALL_N:(k + 1) * WALL_N],
                                    wall_d[k * 128:(k + 1) * 128, :])

            def wall_rhs(k, col0, ncols):
                return wall_sb[:, k * WALL_N + col0: k * WALL_N + col0 + ncols]

            # ---------------- helpers ----------------
            QI, QF, QG, QO = 0, 1, 2, 3

            def bias_mm(p, q, bias2_sb, col0=0, ncols=QH):
                """One K=2 matmul: the indicator-rows lhsT broadcasts the
                per-half gate bias rows into the folded [128, ncols] PSUM
                region, opening its accumulation group."""
                nc.tensor.matmul(
                    p[:, col0:col0 + ncols],
                    fold2_sb[:, :],
                    bias2_sb[:, q * QH + col0: q * QH + col0 + ncols],
                    start=True, stop=False,
                    skip_group_check=True,
                )

            def quarter_mms(p, q, lhs_fn, rhs_fn, kseq, col0=0, ncols=QH,
                            want_stop=True):
                """Gate-quarter matmul pairs into folded PSUM columns
                [col0:col0+ncols]; group must already be open (bias_mm).
                want_stop=False leaves the group open (sub-range splits:
                only the final sub-range's last matmul carries stop)."""
                last = kseq[-1]
                for k in kseq:
                    for half in range(2):
                        c0 = q * H + half * QH + col0
                        nc.tensor.matmul(
                            p[half * 64:(half + 1) * 64, col0:col0 + ncols],
                            lhs_fn(k),
                            rhs_fn(k, c0, ncols),
                            start=False,
                            stop=(want_stop and k == last),
                            skip_group_check=True,
                        )

            def act(func, out_t, in_t):
                nc.scalar.activation(out_t[:], in_t[:], func)

            def act_cols(func, out_t, in_t, c0, nc_):
                nc.scalar.activation(out_t[:, c0:c0 + nc_], in_t[:, c0:c0 + nc_],
                                     func)

            def ht_chunk(hT, k):
                col0 = (k % 4) * 128 + (k // 4) * 64
                return hT[:, col0:col0 + 64]

            def fc_mms(hT_prev):
                # fc bias is added on the host after the gather
                p = pfc.tile([64, O], f32, tag="pfc")
                for k in range(KCH):
                    nc.tensor.matmul(
                        p[:, :],
                        ht_chunk(hT_prev, k),
                        wall_rhs(k, G4, O),
                        start=(k == 0), stop=(k == KCH - 1),
                    )
                return p

            def fc_out(p, t_row):
                sb = fcpool.tile([64, O], f32, tag="fcsb")
                nc.vector.tensor_copy(sb[:], p[:])
                if tiny_out:
                    if t_row == T_steps - 1:
                        nc.sync.dma_start(out_d[:, 0, :], sb[:])
                else:
                    nc.sync.dma_start(out_d[:, t_row, :], sb[:])

            def transpose_blocks(h_both, p, blocks):
                """PE transposes of folded-h column blocks into ptr PSUM.
                Block j covers h cols [j*128,(j+1)*128) -> hT chunks j, j+4."""
                for j in blocks:
                    nc.tensor.matmul(
                        p[:, j * 128:(j + 1) * 128],
                        h_both[:, j * 128:(j + 1) * 128],
                        ident_sb[:, :],
                        is_transpose=True, start=True, stop=True,
                    )

            # ---------------- encoder ----------------
            # gates_e = x @ enc_Wih.T + enc_b ; f-gate unused (c_prev = 0)
            def enc_lhs(k):
                return xT_sb[:, k * BQ:(k + 1) * BQ]

            def enc_rhs(k, col0, ncols):
                return encW_sb[:, k * G4 + col0: k * G4 + col0 + ncols]

            p_g = pg.tile([128, QH], f32, tag="pg", name="enc_g")
            bias_mm(p_g, QG, encb2_sb)
            quarter_mms(p_g, QG, enc_lhs, enc_rhs, (0, 1))
            s_g = sgpool.tile([128, QH], f32, tag="s_g")
            act(AF.Tanh, s_g, p_g)
            p_i = pg.tile([128, QH], f32, tag="pg", name="enc_i")
            bias_mm(p_i, QI, encb2_sb)
            quarter_mms(p_i, QI, enc_lhs, enc_rhs, (0, 1))
            s_i = sgpool.tile([128, QH], f32, tag="s_i")
            act(AF.Sigmoid, s_i, p_i)
            c_cur = cpool.tile([128, QH], f32, tag="c")
            nc.vector.tensor_mul(c_cur[:], s_i[:], s_g[:])
            tc_t = ttpool.tile([128, QH], f32, tag="tc")
            act(AF.Tanh, tc_t, c_cur)
            p_o = po.tile([128, QH], f32, tag="po", name="enc_o")
            bias_mm(p_o, QO, encb2_sb)
            quarter_mms(p_o, QO, enc_lhs, enc_rhs, (0, 1))
            s_o = sgpool.tile([128, QH], f32, tag="s_o")
            act(AF.Sigmoid, s_o, p_o)
            h_both = hpool.tile([128, QH], bf16, tag="h")
            nc.vector.tensor_mul(h_both[:], s_o[:], tc_t[:])
            p_tr = ptr.tile([128, KCH * 64], bf16, tag="ptr", name="enc_tr")
            transpose_blocks(h_both, p_tr, (0, 1, 2, 3))
            hT_cur = htpool.tile([128, KCH * 64], bf16, tag="hT",
                                 name="enc_hT")
            nc.vector.tensor_copy(hT_cur[:], p_tr[:])

            # ---------------- decoder step 0 ----------------
            # gates_0 = h_enc @ dec_Whh.T + dec_b, streaming whhT from HBM in
            # k-chunk pairs; k-outer so each streamed chunk is used once.
            hT_prev = hT_cur
            c_prev = c_cur
            pq = [(po if q_ == QO else pg).tile(
                [128, QH], f32, tag=("po" if q_ == QO else "pg"),
                name=f"t0_q{q_}") for q_ in range(4)]
            for q in range(4):
                bias_mm(pq[q], q, decb2_sb)
            for pair in range(KCH // 2):
                wt = wtmp.tile([128, 2 * G4], bf16, tag="wt")
                for kk in range(2):
                    k = 2 * pair + kk
                    nc.sync.dma_start(
                        wt[:, kk * G4:(kk + 1) * G4],
                        whhT_d[k * 128:(k + 1) * 128, :])
                for kk in range(2):
                    k = 2 * pair + kk
                    last = k == KCH - 1
                    for q in range(4):
                        for half in range(2):
                            col0 = q * H + half * QH
                            nc.tensor.matmul(
                                pq[q][half * 64:(half + 1) * 64, :],
                                ht_chunk(hT_prev, k),
                                wt[:, kk * G4 + col0: kk * G4 + col0 + QH],
                                start=False, stop=last,
                                skip_group_check=True,
                            )
            p_g, p_i, p_f, p_o = pq[QG], pq[QI], pq[QF], pq[QO]
            s_g = sgpool.tile([128, QH], f32, tag="s_g")
            act(AF.Tanh, s_g, p_g)
            s_i = sgpool.tile([128, QH], f32, tag="s_i")
            act(AF.Sigmoid, s_i, p_i)
            s_f = sgpool.tile([128, QH], f32, tag="s_f")
            act(AF.Sigmoid, s_f, p_f)
            t2 = ttpool.tile([128, QH], f32, tag="t2")
            nc.vector.tensor_mul(t2[:], s_i[:], s_g[:])
            t1 = ttpool.tile([128, QH], f32, tag="t1")
            nc.vector.tensor_mul(t1[:], s_f[:], c_prev[:])
            c_cur = cpool.tile([128, QH], f32, tag="c")
            nc.vector.tensor_add(c_cur[:], t1[:], t2[:])
            tc_t = ttpool.tile([128, QH], f32, tag="tc")
            act(AF.Tanh, tc_t, c_cur)
            s_o = sgpool.tile([128, QH], f32, tag="s_o")
            act(AF.Sigmoid, s_o, p_o)
            h_both = hpool.tile([128, QH], bf16, tag="h")
            nc.vector.tensor_mul(h_both[:], s_o[:], tc_t[:])
            p_tr = ptr.tile([128, KCH * 64], bf16, tag="ptr", name="t0_tr")
            transpose_blocks(h_both, p_tr, (0, 1, 2, 3))
            hT_cur = htpool.tile([128, KCH * 64], bf16, tag="hT",
                                 name="t0_hT")
            nc.vector.tensor_copy(hT_cur[:], p_tr[:])

            # ---------------- decoder steps 1..T-1 ----------------
            for t in range(1, T_steps):
                hT_prev = hT_cur
                c_prev = c_cur
                # allocate at step start in [g,i,f,o] order: vs the 5-slot
                # ring this gives g<-o(t-2), i<-g(t-1), f<-i(t-1), o<-f(t-1)
                # so no bias WARs on a late-read tile
                p_gq = pg.tile([128, QH], f32, tag="pg", name=f"s{t}_g")
                p_iq = pg.tile([128, QH], f32, tag="pg", name=f"s{t}_i")
                p_fq = pg.tile([128, QH], f32, tag="pg", name=f"s{t}_f")
                p_oq = po.tile([128, QH], f32, tag="po", name=f"s{t}_o")

                def dec_lhs(k, _h=hT_prev):
                    return ht_chunk(_h, k)

                # --- PE: gate quarters (g, i full; f, o split A/B) ---
                bias_mm(p_gq, QG, decb2_sb)
                quarter_mms(p_gq, QG, dec_lhs, wall_rhs, K_ORDER)
                bias_mm(p_iq, QI, decb2_sb)
                quarter_mms(p_iq, QI, dec_lhs, wall_rhs, K_ORDER)
                bias_mm(p_fq, QF, decb2_sb)
                quarter_mms(p_fq, QF, dec_lhs, wall_rhs, K_ORDER, 0, QA,
                            want_stop=False)
                quarter_mms(p_fq, QF, dec_lhs, wall_rhs, K_ORDER, QA, QA)
                bias_mm(p_oq, QO, decb2_sb)
                quarter_mms(p_oq, QO, dec_lhs, wall_rhs, K_ORDER, 0, QA,
                            want_stop=False)
                quarter_mms(p_oq, QO, dec_lhs, wall_rhs, K_ORDER, QA, QA)

                # --- ACT queue: gate activations (explicit order) ---
                s_g = sgpool.tile([128, QH], f32, tag="s_g")
                act(AF.Tanh, s_g, p_gq)
                s_i = sgpool.tile([128, QH], f32, tag="s_i")
                act(AF.Sigmoid, s_i, p_iq)
                s_f = sgpool.tile([128, QH], f32, tag="s_f")
                act_cols(AF.Sigmoid, s_f, p_fq, 0, QA)
                act_cols(AF.Sigmoid, s_f, p_fq, QA, QA)
                s_o = sgpool.tile([128, QH], f32, tag="s_o")
                tc_t = ttpool.tile([128, QH], f32, tag="tc")
                t2 = ttpool.tile([128, QH], f32, tag="t2")
                t1 = ttpool.tile([128, QH], f32, tag="t1")
                c_cur = cpool.tile([128, QH], f32, tag="c")
                h_both = hpool.tile([128, QH], bf16, tag="h")
                # c-chain on DVE, activations on ACT
                nc.vector.tensor_mul(t2[:, 0:QA], s_i[:, 0:QA], s_g[:, 0:QA])
                nc.vector.tensor_mul(t1[:, 0:QA], s_f[:, 0:QA],
                                     c_prev[:, 0:QA])
                nc.vector.tensor_add(c_cur[:, 0:QA], t1[:, 0:QA], t2[:, 0:QA])
                act_cols(AF.Tanh, tc_t, c_cur, 0, QA)
                act_cols(AF.Sigmoid, s_o, p_oq, 0, QA)
                nc.vector.tensor_mul(t2[:, QA:QH], s_i[:, QA:QH],
                                     s_g[:, QA:QH])
                nc.vector.tensor_mul(t1[:, QA:QH], s_f[:, QA:QH],
                                     c_prev[:, QA:QH])
                nc.vector.tensor_add(c_cur[:, QA:QH], t1[:, QA:QH],
                                     t2[:, QA:QH])
                act_cols(AF.Tanh, tc_t, c_cur, QA, QA)
                act_cols(AF.Sigmoid, s_o, p_oq, QA, QA)
                nc.vector.tensor_mul(h_both[:, 0:QA], s_o[:, 0:QA],
                                     tc_t[:, 0:QA])
                nc.vector.tensor_mul(h_both[:, QA:QH], s_o[:, QA:QH],
                                     tc_t[:, QA:QH])

                # --- PE tail: fc(t-1) ride, transposes, next-step biases ---
                p = fc_mms(hT_prev)
                p_tr = ptr.tile([128, KCH * 64], bf16, tag="ptr",
                                name=f"tr{t}")
                transpose_blocks(h_both, p_tr, (0, 1))

                transpose_blocks(h_both, p_tr, (2, 3))

                # hT copies: block 0 then block 1 on ACT (fine-grained so
                # the next step's first gate pairs unblock after ~1 block),
                # blocks 2,3 on DVE
                hT_cur = htpool.tile([128, KCH * 64], bf16, tag="hT",
                                     name=f"hT{t}")
                nc.scalar.activation(hT_cur[:, 0:128], p_tr[:, 0:128], AF.Copy)
                nc.scalar.activation(hT_cur[:, 128:256], p_tr[:, 128:256],
                                     AF.Copy)
                nc.vector.tensor_copy(hT_cur[:, 256:512], p_tr[:, 256:512])
                fc_out(p, t - 1)

            # fc epilogue for the last step's h
            p = fc_mms(hT_cur)
            fc_out(p, T_steps - 1)

    nc.compile()
    return nc


def _prep_inputs(x, enc_Wih, enc_Whh, enc_bih, enc_bhh,
                 dec_Wih, dec_Whh, dec_bih, dec_bhh, fc_W, fc_b):
    """Host-side prep: fuse/transpose/cast; returns per-core in_maps."""
    x = np.asarray(x, _F32)
    wc = np.asarray(dec_Wih, _F32) + np.asarray(dec_Whh, _F32)  # [4H, H]
    wall = np.concatenate([wc.T, np.asarray(fc_W, _F32).T], axis=1)  # [H, 4H+O]
    whhT = np.ascontiguousarray(np.asarray(dec_Whh, _F32).T)  # [H, 4H]
    encW = np.ascontiguousarray(np.asarray(enc_Wih, _F32).T)  # [I, 4H]
    encb = np.asarray(enc_bih, _F32) + np.asarray(enc_bhh, _F32)
    decb = np.asarray(dec_bih, _F32) + np.asarray(dec_bhh, _F32)

    def stack2(b):
        # [2, 2048]: row0 = per-quarter first halves, row1 = second halves
        q = b.reshape(4, 2, 512)
        return np.stack([q[:, 0, :].reshape(-1), q[:, 1, :].reshape(-1)])

    encb2 = stack2(encb)
    decb2 = stack2(decb)
    xT = np.ascontiguousarray(x.T)  # [I, B]
    ident = np.eye(128, dtype=_F32)
    ones = np.ones((1, BQ), _F32)
    fold2 = np.zeros((2, 128), _F32)
    fold2[0, 0:64] = 1.0
    fold2[1, 64:128] = 1.0

    shared = {
        "encW": _bf16(encW),
        "whhT": _bf16(whhT),
        "wall": _bf16(wall),
        "encb2": _bf16(encb2),
        "decb2": _bf16(decb2),
        "ones": _bf16(ones),
        "fold2": _bf16(fold2),
        "ident": _bf16(ident),
    }
    in_maps = []
    for c in range(N_CORES):
        m = dict(shared)
        m["xT"] = _bf16(xT[:, c * BQ:(c + 1) * BQ])
        in_maps.append(m)
    return in_maps


_CACHED = {}


def _get_compiled(T_steps=T):
    if T_steps not in _CACHED:
        _CACHED[T_steps] = build_bass(T_steps)
    return _CACHED[T_steps]


def kernel(**inputs):
    from concourse.bass_utils import run_bass_kernel_spmd

    nc = _get_compiled(T)
    in_maps = _prep_inputs(**inputs)
    res = run_bass_kernel_spmd(nc, in_maps, core_ids=list(range(N_CORES)))
    outs = [res.results[c]["out"] for c in range(N_CORES)]
    out = np.concatenate(outs, axis=0)  # [B, T, O] fp32
    out += np.asarray(inputs["fc_b"], _F32)[None, None, :]
    return out


if __name__ == "__main__":
    # quick shape smoke test with random inputs
    rng = np.random.default_rng(0)
    ins = {
        "x": rng.standard_normal((B, I), dtype=_F32),
        "enc_Wih": rng.standard_normal((G4, I), dtype=_F32) * 0.03,
        "enc_Whh": rng.standard_normal((G4, H), dtype=_F32) * 0.03,
        "enc_bih": rng.standard_normal(G4).astype(_F32) * 0.03,
        "enc_bhh": rng.standard_normal(G4).astype(_F32) * 0.03,
        "dec_Wih": rng.standard_normal((G4, H), dtype=_F32) * 0.03,
        "dec_Whh": rng.standard_normal((G4, H), dtype=_F32) * 0.03,
        "dec_bih": rng.standard_normal(G4).astype(_F32) * 0.03,
        "dec_bhh": rng.standard_normal(G4).astype(_F32) * 0.03,
        "fc_W": rng.standard_normal((O, H), dtype=_F32) * 0.03,
        "fc_b": rng.standard_normal(O).astype(_F32) * 0.03,
    }
    out = kernel(**ins)
    print("out", out.shape, out.dtype, float(np.abs(out).mean()))


# revision 33
# speedup vs baseline: 1.1933x; 1.0016x over previous
"""Trainium2 Bass kernel for a seq2seq LSTM (1-step encoder + T-step decoder + FC).

Model (B=512, I=256, H=1024, O=128, T=100):
  h,c   = LSTMCell(x, 0, 0; enc_Wih, enc_Whh, enc_b)          # encoder
  loop t in 0..T-1:  h,c = LSTMCell(dec_in, h, c; dec_*)      # decoder
      where dec_in == 0 for t==0 and dec_in == h (same tensor!) for t>=1
  out[:, t, :] = h_t @ fc_W.T + fc_b

Key algebraic fusion: for t>=1 the cell input equals the hidden state, so
  gates_t = h_{t-1} @ (dec_Wih + dec_Whh).T + dec_b
and for t==0 (dec_in = 0):
  gates_0 = h_enc @ dec_Whh.T + dec_b

Sharding: pure data-parallel over batch across 8 NeuronCores (64 rows each),
weights replicated.

Performance design (v2): the TRN2 PE clock p-states run at 1.2 GHz until the
engine has executed continuously for ~3 us, reaching 2.4 GHz only while it
stays busy; any long idle gap resets it.  The per-step schedule therefore
keeps the PE stream gapless:
  - matmuls: out = lhsT.T @ rhs with lhsT = transposed hidden state
    (hT, [128 x 64] per 128-hidden chunk), rhs = pre-transposed weights.
    PE column-pair tiling: two concurrent M=64 matmuls at (0,0)/(0,64)
    compute the two hidden-halves of each gate quarter into a folded
    [128, 512] PSUM tile (batch duplicated across partition halves).
  - each quarter's bias is ONE K=2 matmul (indicator-rows lhsT) opening the
    PSUM accumulation group.
  - the f and o quarters are split into two N=256 accumulation groups so the
    c-chain (sig f -> f*c + i*g -> tanh -> h) pipelines in column halves.
  - per-step PE stream: bias/gate matmuls for g,i,fA,fB,oA,oB, then the
    fc(t-1) ride and the h transposes; the Tile scheduler (greedy by
    emission priority over a compile-time timing sim) hoists next-step
    biases into the tail.  Gate k-order {0,4,1,5,2,6,3,7} consumes the hT
    copies as they complete (blocks 0,1 copied on ACT, 2,3 on DVE).
  - _calibrate_cost_model() slows the build-time model's ACT/DVE fixed
    overheads to hardware-measured values: the frozen per-engine order is
    enforced with cross-engine waits, so an optimistic model makes the PE
    stall on activations that finish later than the scheduler predicted.
  - fc bias is folded in on the host after the gather (saves a PE matmul
    per step); per-quarter PSUM pools are sized so no accumulation-group
    open ever WAR-waits on a late activation read.
"""

import os
import sys

import numpy as np

_TRN_REPO = "/opt/trn_rl_repo"
if _TRN_REPO not in sys.path:
    sys.path.insert(0, _TRN_REPO)

B, I, H, O, T = 512, 256, 1024, 128, 100
N_CORES = 8
BQ = B // N_CORES  # 64 batch rows per core
KCH = H // 128     # 8 k-chunks of the hidden dim
G4 = 4 * H         # 4096 gate columns
WALL_N = G4 + O    # gate weights + fc weights, concatenated along columns

_F32 = np.float32

# gate k-chunk order: consume hT transpose blocks 0,1 then 2,3
K_ORDER = (0, 4, 1, 5, 2, 6, 3, 7)


def _bf16(a):
    import ml_dtypes

    return np.asarray(a, dtype=ml_dtypes.bfloat16)


_CALIBRATED = False


def _calibrate_cost_model():
    """Calibrate the build-time cost model's ACT/DVE fixed overheads to the
    values measured on hardware (the stock spec is ~150-250 ns/op optimistic
    for the Activation engine).  The Tile scheduler freezes each engine's
    instruction order against this model, and enforces that order with
    cross-engine waits; an optimistic ACT model makes the frozen PE order
    stall on activations that finish later than predicted.  Must run before
    the first compile in the process (the Rust cost model caches the spec).
    """
    global _CALIBRATED
    if _CALIBRATED:
        return
    _CALIBRATED = True
    import concourse.bass as bass
    from concourse import hw_specs, mybir

    ac = dict(hw_specs.TRN2Spec.ACCESS_CYCLES)
    ac[(bass.MemorySpace.SBUF, mybir.EngineType.Activation)] = 470
    ac[(bass.MemorySpace.PSUM, mybir.EngineType.Activation)] = 420
    ac[(bass.MemorySpace.PSUM, mybir.EngineType.DVE)] = 160
    hw_specs.TRN2Spec.ACCESS_CYCLES = ac


def build_bass(T_steps=T, tiny_out=False):
    """Builds the per-core Bass program (same program on all 8 cores)."""
    import concourse.bass as bass
    import concourse.tile as tile
    from concourse import bacc, mybir

    _calibrate_cost_model()

    f32 = mybir.dt.float32
    bf16 = mybir.dt.bfloat16
    AF = mybir.ActivationFunctionType

    nc = bacc.Bacc("TRN2", target_bir_lowering=False, debug=False,
                   enable_asserts=False)

    # ---- DRAM I/O ----
    xT_d = nc.dram_tensor("xT", [I, BQ], bf16, kind="ExternalInput").ap()
    encW_d = nc.dram_tensor("encW", [I, G4], bf16, kind="ExternalInput").ap()
    whhT_d = nc.dram_tensor("whhT", [H, G4], bf16, kind="ExternalInput").ap()
    wall_d = nc.dram_tensor("wall", [H, WALL_N], bf16, kind="ExternalInput").ap()
    encb2_d = nc.dram_tensor("encb2", [2, G4 // 2], bf16, kind="ExternalInput").ap()
    decb2_d = nc.dram_tensor("decb2", [2, G4 // 2], bf16, kind="ExternalInput").ap()
    ones_d = nc.dram_tensor("ones", [1, BQ], bf16, kind="ExternalInput").ap()
    fold2_d = nc.dram_tensor("fold2", [2, 128], bf16, kind="ExternalInput").ap()
    ident_d = nc.dram_tensor("ident", [128, 128], bf16, kind="ExternalInput").ap()
    out_T = 1 if tiny_out else T_steps
    out_d = nc.dram_tensor("out", [BQ, out_T, O], f32, kind="ExternalOutput").ap()

    QH = 512  # hidden half (columns per folded tile)
    QA = 256  # split-quarter group width

    with tile.TileContext(nc) as tc:
        from contextlib import ExitStack

        ctx = ExitStack()
        with ctx:
            # ---- persistent SBUF pools ----
            consts = ctx.enter_context(tc.tile_pool(name="consts", bufs=1))
            wpool = ctx.enter_context(tc.tile_pool(name="wpool", bufs=1))
            wtmp = ctx.enter_context(tc.tile_pool(name="wtmp", bufs=2))
            cpool = ctx.enter_context(tc.tile_pool(name="cpool", bufs=2))
            hpool = ctx.enter_context(tc.tile_pool(name="hpool", bufs=2))
            htpool = ctx.enter_context(tc.tile_pool(name="htpool", bufs=2))
            sgpool = ctx.enter_context(tc.tile_pool(name="sgpool", bufs=2))
            ttpool = ctx.enter_context(tc.tile_pool(name="ttpool", bufs=2))
            fcpool = ctx.enter_context(tc.tile_pool(name="fcpool", bufs=3))
            # PSUM pools (8 banks total: 3 + 2 + 1 + 2).  The o quarter
            # gets its own 2-buffer pool: its bias (opened in the step tail)
            # must never WAR-wait on the just-completed sig_o of this step.
            pg = ctx.enter_context(tc.tile_pool(name="pg", bufs=3, space="PSUM"))
            po = ctx.enter_context(tc.tile_pool(name="po", bufs=2, space="PSUM"))
            ptr = ctx.enter_context(tc.tile_pool(name="ptr", bufs=1, space="PSUM"))
            pfc = ctx.enter_context(tc.tile_pool(name="pfc", bufs=2, space="PSUM"))

            # ---- constants / weights into SBUF ----
            ones_sb = consts.tile([1, BQ], bf16, tag="ones")
            nc.sync.dma_start(ones_sb[:], ones_d[:])
            fold2_sb = consts.tile([2, 128], bf16, tag="fold2")
            nc.sync.dma_start(fold2_sb[:], fold2_d[:])
            ident_sb = consts.tile([128, 128], bf16, tag="ident")
            nc.sync.dma_start(ident_sb[:], ident_d[:])
            encb2_sb = consts.tile([2, G4 // 2], bf16, tag="encb2")
            nc.sync.dma_start(encb2_sb[:], encb2_d[:])
            decb2_sb = consts.tile([2, G4 // 2], bf16, tag="decb2")
            nc.sync.dma_start(decb2_sb[:], decb2_d[:])
            xT_sb = consts.tile([128, 2 * BQ], bf16, tag="xT")
            for k in range(2):
                nc.sync.dma_start(xT_sb[:, k * BQ:(k + 1) * BQ],
                                  xT_d[k * 128:(k + 1) * 128, :])
            encW_sb = consts.tile([128, 2 * G4], bf16, tag="encW")
            for k in range(2):
                nc.sync.dma_start(encW_sb[:, k * G4:(k + 1) * G4],
                                  encW_d[k * 128:(k + 1) * 128, :])

            # main fused weights, resident: [128, KCH * WALL_N] bf16
            # (issued on the gpsimd queue so the whhT stream on the sync
            # queue is not serialized behind it)
            wall_sb = wpool.tile([128, KCH * WALL_N], bf16, tag="wall")
            for k in range(KCH):
                nc.gpsimd.dma_start(wall_sb[:, k * WALL_N:(k + 1) * WALL_N],
                                    wall_d[k * 128:(k + 1) * 128, :])

            def wall_rhs(k, col0, ncols):
                return wall_sb[:, k * WALL_N + col0: k * WALL_N + col0 + ncols]

            # ---------------- helpers ----------------
            QI, QF, QG, QO = 0, 1, 2, 3

            def bias_mm(p, q, bias2_sb, col0=0, ncols=QH):
                """One K=2 matmul: the indicator-rows lhsT broadcasts the
                per-half gate bias rows into the folded [128, ncols] PSUM
                region, opening its accumulation group."""
                nc.tensor.matmul(
                    p[:, col0:col0 + ncols],
                    fold2_sb[:, :],
                    bias2_sb[:, q * QH + col0: q * QH + col0 + ncols],
                    start=True, stop=False,
                    skip_group_check=True,
                )

            def quarter_mms(p, q, lhs_fn, rhs_fn, kseq, col0=0, ncols=QH,
                            want_stop=True):
                """Gate-quarter matmul pairs into folded PSUM columns
                [col0:col0+ncols]; group must already be open (bias_mm).
                want_stop=False leaves the group open (sub-range splits:
                only the final sub-range's last matmul carries stop)."""
                last = kseq[-1]
                for k in kseq:
                    for half in range(2):
                        c0 = q * H + half * QH + col0
                        nc.tensor.matmul(
                            p[half * 64:(half + 1) * 64, col0:col0 + ncols],
                            lhs_fn(k),
                            rhs_fn(k, c0, ncols),
                            start=False,
                            stop=(want_stop and k == last),
                            skip_group_check=True,
                        )

            def act(func, out_t, in_t):
                nc.scalar.activation(out_t[:], in_t[:], func)

            def act_cols(func, out_t, in_t, c0, nc_):
                nc.scalar.activation(out_t[:, c0:c0 + nc_], in_t[:, c0:c0 + nc_],
                                     func)

            def ht_chunk(hT, k):
                col0 = (k % 4) * 128 + (k // 4) * 64
                return hT[:, col0:col0 + 64]

            def fc_mms(hT_prev):
                # fc bias is added on the host after the gather
                p = pfc.tile([64, O], f32, tag="pfc")
                for k in range(KCH):
                    nc.tensor.matmul(
                        p[:, :],
                        ht_chunk(hT_prev, k),
                        wall_rhs(k, G4, O),
                        start=(k == 0), stop=(k == KCH - 1),
                    )
                return p

            def fc_out(p, t_row):
                sb = fcpool.tile([64, O], f32, tag="fcsb")
                nc.vector.tensor_copy(sb[:], p[:])
                if tiny_out:
                    if t_row == T_steps - 1:
                        nc.sync.dma_start(out_d[:, 0, :], sb[:])
                else:
                    nc.sync.dma_start(out_d[:, t_row, :], sb[:])

            def transpose_blocks(h_both, p, blocks):
                """PE transposes of folded-h column blocks into ptr PSUM.
                Block j covers h cols [j*128,(j+1)*128) -> hT chunks j, j+4."""
                for j in blocks:
                    nc.tensor.matmul(
                        p[:, j * 128:(j + 1) * 128],
                        h_both[:, j * 128:(j + 1) * 128],
                        ident_sb[:, :],
                        is_transpose=True, start=True, stop=True,
                    )

            # ---------------- encoder ----------------
            # gates_e = x @ enc_Wih.T + enc_b ; f-gate unused (c_prev = 0)
            def enc_lhs(k):
                return xT_sb[:, k * BQ:(k + 1) * BQ]

            def enc_rhs(k, col0, ncols):
                return encW_sb[:, k * G4 + col0: k * G4 + col0 + ncols]

            p_g = pg.tile([128, QH], f32, tag="pg", name="enc_g")
            bias_mm(p_g, QG, encb2_sb)
            quarter_mms(p_g, QG, enc_lhs, enc_rhs, (0, 1))
            s_g = sgpool.tile([128, QH], f32, tag="s_g")
            act(AF.Tanh, s_g, p_g)
            p_i = pg.tile([128, QH], f32, tag="pg", name="enc_i")
            bias_mm(p_i, QI, encb2_sb)
            quarter_mms(p_i, QI, enc_lhs, enc_rhs, (0, 1))
            s_i = sgpool.tile([128, QH], f32, tag="s_i")
            act(AF.Sigmoid, s_i, p_i)
            c_cur = cpool.tile([128, QH], f32, tag="c")
            nc.vector.tensor_mul(c_cur[:], s_i[:], s_g[:])
            tc_t = ttpool.tile([128, QH], f32, tag="tc")
            act(AF.Tanh, tc_t, c_cur)
            p_o = po.tile([128, QH], f32, tag="po", name="enc_o")
            bias_mm(p_o, QO, encb2_sb)
            quarter_mms(p_o, QO, enc_lhs, enc_rhs, (0, 1))
            s_o = sgpool.tile([128, QH], f32, tag="s_o")
            act(AF.Sigmoid, s_o, p_o)
            h_both = hpool.tile([128, QH], bf16, tag="h")
            nc.vector.tensor_mul(h_both[:], s_o[:], tc_t[:])
            p_tr = ptr.tile([128, KCH * 64], bf16, tag="ptr", name="enc_tr")
            transpose_blocks(h_both, p_tr, (0, 1, 2, 3))
            hT_cur = htpool.tile([128, KCH * 64], bf16, tag="hT",
                                 name="enc_hT")
            nc.vector.tensor_copy(hT_cur[:], p_tr[:])

            # ---------------- decoder step 0 ----------------
            # gates_0 = h_enc @ dec_Whh.T + dec_b, streaming whhT from HBM in
            # k-chunk pairs; k-outer so each streamed chunk is used once.
            hT_prev = hT_cur
            c_prev = c_cur
            pq = [(po if q_ == QO else pg).tile(
                [128, QH], f32, tag=("po" if q_ == QO else "pg"),
                name=f"t0_q{q_}") for q_ in range(4)]
            for q in range(4):
                bias_mm(pq[q], q, decb2_sb)
            for pair in range(KCH // 2):
                wt = wtmp.tile([128, 2 * G4], bf16, tag="wt")
                for kk in range(2):
                    k = 2 * pair + kk
                    nc.sync.dma_start(
                        wt[:, kk * G4:(kk + 1) * G4],
                        whhT_d[k * 128:(k + 1) * 128, :])
                for kk in range(2):
                    k = 2 * pair + kk
                    last = k == KCH - 1
                    for q in range(4):
                        for half in range(2):
                            col0 = q * H + half * QH
                            nc.tensor.matmul(
                                pq[q][half * 64:(half + 1) * 64, :],
                                ht_chunk(hT_prev, k),
                                wt[:, kk * G4 + col0: kk * G4 + col0 + QH],
                                start=False, stop=last,
                                skip_group_check=True,
                            )
            p_g, p_i, p_f, p_o = pq[QG], pq[QI], pq[QF], pq[QO]
            s_g = sgpool.tile([128, QH], f32, tag="s_g")
            act(AF.Tanh, s_g, p_g)
            s_i = sgpool.tile([128, QH], f32, tag="s_i")
            act(AF.Sigmoid, s_i, p_i)
            s_f = sgpool.tile([128, QH], f32, tag="s_f")
            act(AF.Sigmoid, s_f, p_f)
            t2 = ttpool.tile([128, QH], f32, tag="t2")
            nc.vector.tensor_mul(t2[:], s_i[:], s_g[:])
            t1 = ttpool.tile([128, QH], f32, tag="t1")
            nc.vector.tensor_mul(t1[:], s_f[:], c_prev[:])
            c_cur = cpool.tile([128, QH], f32, tag="c")
            nc.vector.tensor_add(c_cur[:], t1[:], t2[:])
            tc_t = ttpool.tile([128, QH], f32, tag="tc")
            act(AF.Tanh, tc_t, c_cur)
            s_o = sgpool.tile([128, QH], f32, tag="s_o")
            act(AF.Sigmoid, s_o, p_o)
            h_both = hpool.tile([128, QH], bf16, tag="h")
            nc.vector.tensor_mul(h_both[:], s_o[:], tc_t[:])
            p_tr = ptr.tile([128, KCH * 64], bf16, tag="ptr", name="t0_tr")
            transpose_blocks(h_both, p_tr, (0, 1, 2, 3))
            hT_cur = htpool.tile([128, KCH * 64], bf16, tag="hT",
                                 name="t0_hT")
            nc.vector.tensor_copy(hT_cur[:], p_tr[:])

            # ---------------- decoder steps 1..T-1 ----------------
            for t in range(1, T_steps):
                hT_prev = hT_cur
                c_prev = c_cur
                # allocate at step start in [g,i,f,o] order: vs the 5-slot
                # ring this gives g<-o(t-2), i<-g(t-1), f<-i(t-1), o<-f(t-1)
                # so no bias WARs on a late-read tile
                p_gq = pg.tile([128, QH], f32, tag="pg", name=f"s{t}_g")
                p_iq = pg.tile([128, QH], f32, tag="pg", name=f"s{t}_i")
                p_fq = pg.tile([128, QH], f32, tag="pg", name=f"s{t}_f")
                p_oq = po.tile([128, QH], f32, tag="po", name=f"s{t}_o")

                def dec_lhs(k, _h=hT_prev):
                    return ht_chunk(_h, k)

                # --- PE: gate quarters (g, i full; f, o split A/B) ---
                bias_mm(p_gq, QG, decb2_sb)
                quarter_mms(p_gq, QG, dec_lhs, wall_rhs, K_ORDER)
                bias_mm(p_iq, QI, decb2_sb)
                quarter_mms(p_iq, QI, dec_lhs, wall_rhs, K_ORDER)
                bias_mm(p_fq, QF, decb2_sb)
                quarter_mms(p_fq, QF, dec_lhs, wall_rhs, K_ORDER, 0, QA,
                            want_stop=False)
                quarter_mms(p_fq, QF, dec_lhs, wall_rhs, K_ORDER, QA, QA)
                bias_mm(p_oq, QO, decb2_sb)
                quarter_mms(p_oq, QO, dec_lhs, wall_rhs, K_ORDER, 0, QA,
                            want_stop=False)
                quarter_mms(p_oq, QO, dec_lhs, wall_rhs, K_ORDER, QA, QA)

                # --- ACT queue: gate activations (explicit order) ---
                s_g = sgpool.tile([128, QH], f32, tag="s_g")
                act(AF.Tanh, s_g, p_gq)
                s_i = sgpool.tile([128, QH], f32, tag="s_i")
                act(AF.Sigmoid, s_i, p_iq)
                s_f = sgpool.tile([128, QH], f32, tag="s_f")
                act_cols(AF.Sigmoid, s_f, p_fq, 0, QA)
                act_cols(AF.Sigmoid, s_f, p_fq, QA, QA)
                s_o = sgpool.tile([128, QH], f32, tag="s_o")
                tc_t = ttpool.tile([128, QH], f32, tag="tc")
                t2 = ttpool.tile([128, QH], f32, tag="t2")
                t1 = ttpool.tile([128, QH], f32, tag="t1")
                c_cur = cpool.tile([128, QH], f32, tag="c")
                h_both = hpool.tile([128, QH], bf16, tag="h")
                # c-chain on DVE, activations on ACT
                nc.vector.tensor_mul(t2[:, 0:QA], s_i[:, 0:QA], s_g[:, 0:QA])
                nc.vector.tensor_mul(t1[:, 0:QA], s_f[:, 0:QA],
                                     c_prev[:, 0:QA])
                nc.vector.tensor_add(c_cur[:, 0:QA], t1[:, 0:QA], t2[:, 0:QA])
                act_cols(AF.Tanh, tc_t, c_cur, 0, QA)
                act_cols(AF.Sigmoid, s_o, p_oq, 0, QA)
                nc.vector.tensor_mul(t2[:, QA:QH], s_i[:, QA:QH],
                                     s_g[:, QA:QH])
                nc.vector.tensor_mul(t1[:, QA:QH], s_f[:, QA:QH],
                                     c_prev[:, QA:QH])
                nc.vector.tensor_add(c_cur[:, QA:QH], t1[:, QA:QH],
                                     t2[:, QA:QH])
                act_cols(AF.Tanh, tc_t, c_cur, QA, QA)
                act_cols(AF.Sigmoid, s_o, p_oq, QA, QA)
                nc.vector.tensor_mul(h_both[:, 0:QA], s_o[:, 0:QA],
                                     tc_t[:, 0:QA])
                nc.vector.tensor_mul(h_both[:, QA:QH], s_o[:, QA:QH],
                                     tc_t[:, QA:QH])

                # --- PE tail: fc(t-1) ride, transposes, next-step biases ---
                p = fc_mms(hT_prev)
                p_tr = ptr.tile([128, KCH * 64], bf16, tag="ptr",
                                name=f"tr{t}")
                transpose_blocks(h_both, p_tr, (0, 1))

                transpose_blocks(h_both, p_tr, (2, 3))

                # hT copies: block 0 then block 1 on ACT (fine-grained so
                # the next step's first gate pairs unblock after ~1 block),
                # blocks 2,3 on DVE
                hT_cur = htpool.tile([128, KCH * 64], bf16, tag="hT",
                                     name=f"hT{t}")
                nc.scalar.activation(hT_cur[:, 0:128], p_tr[:, 0:128], AF.Copy)
                nc.scalar.activation(hT_cur[:, 128:256], p_tr[:, 128:256],
                                     AF.Copy)
                nc.vector.tensor_copy(hT_cur[:, 256:512], p_tr[:, 256:512])
                fc_out(p, t - 1)

            # fc epilogue for the last step's h
            p = fc_mms(hT_cur)
            fc_out(p, T_steps - 1)

    nc.compile()
    return nc


def _prep_inputs(x, enc_Wih, enc_Whh, enc_bih, enc_bhh,
                 dec_Wih, dec_Whh, dec_bih, dec_bhh, fc_W, fc_b):
    """Host-side prep: fuse/transpose/cast; returns per-core in_maps."""
    x = np.asarray(x, _F32)
    wc = np.asarray(dec_Wih, _F32) + np.asarray(dec_Whh, _F32)  # [4H, H]
    wall = np.concatenate([wc.T, np.asarray(fc_W, _F32).T], axis=1)  # [H, 4H+O]
    whhT = np.ascontiguousarray(np.asarray(dec_Whh, _F32).T)  # [H, 4H]
    encW = np.ascontiguousarray(np.asarray(enc_Wih, _F32).T)  # [I, 4H]
    encb = np.asarray(enc_bih, _F32) + np.asarray(enc_bhh, _F32)
    decb = np.asarray(dec_bih, _F32) + np.asarray(dec_bhh, _F32)

    def stack2(b):
        # [2, 2048]: row0 = per-quarter first halves, row1 = second halves
        q = b.reshape(4, 2, 512)
        return np.stack([q[:, 0, :].reshape(-1), q[:, 1, :].reshape(-1)])

    encb2 = stack2(encb)
    decb2 = stack2(decb)
    xT = np.ascontiguousarray(x.T)  # [I, B]
    ident = np.eye(128, dtype=_F32)
    ones = np.ones((1, BQ), _F32)
    fold2 = np.zeros((2, 128), _F32)
    fold2[0, 0:64] = 1.0
    fold2[1, 64:128] = 1.0

    shared = {
        "encW": _bf16(encW),
        "whhT": _bf16(whhT),
        "wall": _bf16(wall),
        "encb2": _bf16(encb2),
        "decb2": _bf16(decb2),
        "ones": _bf16(ones),
        "fold2": _bf16(fold2),
        "ident": _bf16(ident),
    }
    in_maps = []
    for c in range(N_CORES):
        m = dict(shared)
        m["xT"] = _bf16(xT[:, c * BQ:(c + 1) * BQ])
        in_maps.append(m)
    return in_maps


_CACHED = {}


def _get_compiled(T_steps=T):
    if T_steps not in _CACHED:
        _CACHED[T_steps] = build_bass(T_steps)
    return _CACHED[T_steps]


def kernel(**inputs):
    from concourse.bass_utils import run_bass_kernel_spmd

    nc = _get_compiled(T)
    in_maps = _prep_inputs(**inputs)
    res = run_bass_kernel_spmd(nc, in_maps, core_ids=list(range(N_CORES)))
    outs = [res.results[c]["out"] for c in range(N_CORES)]
    out = np.concatenate(outs, axis=0)  # [B, T, O] fp32
    out += np.asarray(inputs["fc_b"], _F32)[None, None, :]
    return out


if __name__ == "__main__":
    # quick shape smoke test with random inputs
    rng = np.random.default_rng(0)
    ins = {
        "x": rng.standard_normal((B, I), dtype=_F32),
        "enc_Wih": rng.standard_normal((G4, I), dtype=_F32) * 0.03,
        "enc_Whh": rng.standard_normal((G4, H), dtype=_F32) * 0.03,
        "enc_bih": rng.standard_normal(G4).astype(_F32) * 0.03,
        "enc_bhh": rng.standard_normal(G4).astype(_F32) * 0.03,
        "dec_Wih": rng.standard_normal((G4, H), dtype=_F32) * 0.03,
        "dec_Whh": rng.standard_normal((G4, H), dtype=_F32) * 0.03,
        "dec_bih": rng.standard_normal(G4).astype(_F32) * 0.03,
        "dec_bhh": rng.standard_normal(G4).astype(_F32) * 0.03,
        "fc_W": rng.standard_normal((O, H), dtype=_F32) * 0.03,
        "fc_b": rng.standard_normal(O).astype(_F32) * 0.03,
    }
    out = kernel(**ins)
    print("out", out.shape, out.dtype, float(np.abs(out).mean()))
